# revision 2
# baseline (speedup 1.0000x reference)
"""Trainium2 Bass kernel for nn_AngularSymmetry (B=16, M=64, L=6), 8-core data parallel.

Math (per batch b, output row i, summed over j,k in [0,64)):
  G = coords @ coords.T                      (Gram)
  num[i,j,k]  = nsq[i] - G[i,j] - G[i,k] + G[j,k]     (= vec_ij . vec_ik)
  denp[i,j,k] = (sq2pi*d[i,j])*(sq2pi*d[i,k]) + 2pi*1e-5   (= 2pi*(R_ij R_ik + 1e-5))
  phase = num/denp  (= theta/2pi);  c = cos(2pi*phase) via round-shifted range
  reduction and ACT Sin (valid domain [-pi, pi])
  E[i,j,k] = s[i,j]*s[i,k]*s[j,k],  s = exp(-4 d^2)*d_cutoff
  res[i,l] = 2^(1-zeta_l) * sum_jk (1 + lambda_l*c)^zeta_l * E
with (lambda, zeta) = (+1,2),(+1,4),(+1,8),(-1,2),(-1,4),(-1,8).

Each of the 8 cores handles 2 batches (128 partitions = 2*64 (b,i) rows).
"""
import sys

sys.path.insert(0, "/opt/trn_rl_repo")
import contextlib

import numpy as np

import concourse.bass as bass
import concourse.tile as tile
from concourse import bacc, mybir
from concourse.bass_utils import run_bass_kernel_spmd

F32 = mybir.dt.float32
Alu = mybir.AluOpType
Act = mybir.ActivationFunctionType

B, M, L = 16, 64, 6
NCORES = 8
BPC = B // NCORES  # batches per core = 2
P = BPC * M  # 128 partitions
TWO_PI = float(2.0 * np.pi)
SQ2PI = float(np.sqrt(2.0 * np.pi))
MAGIC = 12582912.0  # 1.5 * 2^23 -> fp32 round-to-int via add/sub
EPS2PI = float(2.0 * np.pi * 1e-5)
NCH = 2  # chunks over k
KC = M // NCH  # k per chunk
GRID = M * KC  # elements per chunk per partition

_NC = None


def _build():
    nc = bacc.Bacc("TRN2", target_bir_lowering=False, debug=False, num_devices=NCORES)
    dcut = nc.dram_tensor("d_cutoff", [BPC, M, M], F32, kind="ExternalInput").ap()
    dd = nc.dram_tensor("d", [BPC, M, M], F32, kind="ExternalInput").ap()
    co = nc.dram_tensor("atom_coordinates", [BPC, M, 3], F32, kind="ExternalInput").ap()
    out = nc.dram_tensor("out", [BPC, M, L], F32, kind="ExternalOutput").ap()
    g_dram = nc.dram_tensor("g_scratch", [BPC, M, M], F32, kind="Internal").ap()
    q_dram = nc.dram_tensor("q_scratch", [BPC, M, M], F32, kind="Internal").ap()

    with tile.TileContext(nc) as tc:
        with contextlib.ExitStack() as ctx:
            pool = ctx.enter_context(tc.tile_pool(name="w", bufs=1))
            psp = ctx.enter_context(tc.tile_pool(name="ps", bufs=1, space="PSUM"))

            # ---------- prep: small [128, <=64] tiles ----------
            d_t = pool.tile([P, M], F32, tag="d_t")
            dc_t = pool.tile([P, M], F32, tag="dc_t")
            co_t = pool.tile([P, 3], F32, tag="co_t")
            ct3 = pool.tile([3, P], F32, tag="ct3")
            nc.sync.dma_start(d_t[:], dd.rearrange("b i j -> (b i) j"))
            nc.sync.dma_start(dc_t[:], dcut.rearrange("b i j -> (b i) j"))
            nc.sync.dma_start(co_t[:], co.rearrange("b i d -> (b i) d"))
            nc.sync.dma_start(ct3[:], co.rearrange("b i d -> d (b i)"))

            # cross-batch Gram [128,128]: out[p,q] = coords_p . coords_q
            gram_ps = psp.tile([P, P], F32, tag="gram")
            nc.tensor.matmul(gram_ps[:], ct3[:], ct3[:], start=True, stop=True)
            g_sb = pool.tile([P, M], F32, tag="g_sb")
            nc.scalar.copy(g_sb[0:M, :], gram_ps[0:M, 0:M])
            nc.scalar.copy(g_sb[M:P, :], gram_ps[M:P, M:P])

            # nsq[p] = |coords_p|^2
            sq3 = pool.tile([P, 3], F32, tag="sq3")
            nc.scalar.square(sq3[:], co_t[:])
            nsq = pool.tile([P, 1], F32, tag="nsq")
            nc.vector.tensor_reduce(nsq[:], sq3[:], axis=mybir.AxisListType.X, op=Alu.add)

            # t1'[p,j] = G[p,j] - nsq[p]
            t1p = pool.tile([P, M], F32, tag="t1p")
            nc.vector.tensor_scalar(t1p[:], g_sb[:], nsq[:], None, op0=Alu.subtract)

            # ut = sqrt(2pi)*d ; s = exp(-4 d^2)*d_cutoff
            ut = pool.tile([P, M], F32, tag="ut")
            nc.vector.tensor_scalar(ut[:], d_t[:], SQ2PI, None, op0=Alu.mult)
            d2 = pool.tile([P, M], F32, tag="d2")
            nc.scalar.square(d2[:], d_t[:])
            e1 = pool.tile([P, M], F32, tag="e1")
            nc.scalar.activation(e1[:], d2[:], Act.Exp, scale=-4.0)
            s_t = pool.tile([P, M], F32, tag="s_t")
            nc.vector.tensor_tensor(s_t[:], e1[:], dc_t[:], op=Alu.mult)

            half_pi = pool.tile([P, 1], F32, tag="half_pi")
            nc.vector.memset(half_pi[:], float(np.pi / 2.0))
            acc = pool.tile([P, L * NCH], F32, tag="acc")
            nc.vector.memset(acc[:], 0.0)

            # stage per-batch G and s(=Q) matrices to DRAM for replication reads
            for b in range(BPC):
                nc.sync.dma_start(g_dram[b], g_sb[b * M : (b + 1) * M, :])
                nc.sync.dma_start(q_dram[b], s_t[b * M : (b + 1) * M, :])

            # ---------- main: 2 chunks over k ----------
            SCALES = [1.0 / 2.0, 1.0 / 8.0, 1.0 / 128.0]  # 2^(1-zeta)

            for ch in range(NCH):
                k0 = ch * KC

                def jb(t):
                    return t[:, :].unsqueeze(2).broadcast_to([P, M, KC])

                def kb(t):
                    return t[:, k0 : k0 + KC].unsqueeze(1).broadcast_to([P, M, KC])

                gb = pool.tile([P, M, KC], F32, tag="gb")
                qq = pool.tile([P, M, KC], F32, tag="qq")
                for b in range(BPC):
                    src = g_dram[b : b + 1, :, k0 : k0 + KC].broadcast_to([M, M, KC])
                    nc.sync.dma_start(gb[b * M : (b + 1) * M, :, :], src)
                    srcq = q_dram[b : b + 1, :, k0 : k0 + KC].broadcast_to([M, M, KC])
                    nc.sync.dma_start(qq[b * M : (b + 1) * M, :, :], srcq)

                num1 = pool.tile([P, M, KC], F32, tag="num1")
                nc.vector.tensor_tensor(num1[:], jb(t1p), kb(g_sb), op=Alu.add)
                num = pool.tile([P, M, KC], F32, tag="num")
                nc.vector.tensor_tensor(num[:], gb[:], num1[:], op=Alu.subtract)

                den = pool.tile([P, M, KC], F32, tag="den")
                nc.vector.tensor_tensor(den[:], jb(ut), kb(ut), op=Alu.mult)
                nc.vector.tensor_scalar(den[:], den[:], EPS2PI, None, op0=Alu.add)
                rec = pool.tile([P, M, KC], F32, tag="rec")
                recs = pool.tile([P, M, KC], F32, tag="recs")
                nc.vector.reciprocal_approx_accurate(rec[:], den[:], recs[:])
                ph = pool.tile([P, M, KC], F32, tag="ph")
                nc.vector.tensor_tensor(ph[:], num[:], rec[:], op=Alu.mult)

                # n = round(ph - 0.25); frn = n - ph  (in [-0.75, 0.25])
                nr = pool.tile([P, M, KC], F32, tag="nr")
                nc.vector.tensor_scalar(nr[:], ph[:], -0.25, MAGIC, op0=Alu.add, op1=Alu.add)
                nc.vector.tensor_scalar(nr[:], nr[:], -MAGIC, None, op0=Alu.add)
                frn = pool.tile([P, M, KC], F32, tag="frn")
                nc.gpsimd.tensor_tensor(frn[:], nr[:], ph[:], op=Alu.subtract)

                # c = cos(2pi*fr) = sin(2pi*frn + pi/2)
                c = pool.tile([P, M, KC], F32, tag="c")
                nc.scalar.activation(c[:], frn[:], Act.Sin, bias=half_pi[:], scale=TWO_PI)

                p2 = pool.tile([P, M, KC], F32, tag="p2")
                nc.scalar.activation(p2[:], c[:], Act.Square, bias=1.0, scale=1.0)
                m2 = pool.tile([P, M, KC], F32, tag="m2")
                nc.scalar.activation(m2[:], c[:], Act.Square, bias=1.0, scale=-1.0)
                p4 = pool.tile([P, M, KC], F32, tag="p4")
                nc.scalar.square(p4[:], p2[:])
                m4 = pool.tile([P, M, KC], F32, tag="m4")
                nc.scalar.square(m4[:], m2[:])

                e0 = pool.tile([P, M, KC], F32, tag="e0")
                nc.vector.tensor_tensor(e0[:], jb(s_t), kb(s_t), op=Alu.mult)
                ee = pool.tile([P, M, KC], F32, tag="ee")
                nc.gpsimd.tensor_tensor(ee[:], e0[:], qq[:], op=Alu.mult)

                # power chains + fused scaled reduces (ACT Copy w/ scale + accum)
                def chain(pw2, pw4, base_l):
                    a2 = pool.tile([P, M, KC], F32, tag=f"a2_{base_l}")
                    nc.vector.tensor_tensor(a2[:], pw2[:], ee[:], op=Alu.mult)
                    a4 = pool.tile([P, M, KC], F32, tag=f"a4_{base_l}")
                    nc.vector.tensor_tensor(a4[:], pw2[:], a2[:], op=Alu.mult)
                    a8 = pool.tile([P, M, KC], F32, tag=f"a8_{base_l}")
                    nc.vector.tensor_tensor(a8[:], pw4[:], a4[:], op=Alu.mult)
                    for idx, a in enumerate([a2, a4, a8]):
                        dst = acc[:, (base_l + idx) * NCH + ch : (base_l + idx) * NCH + ch + 1]
                        red = pool.tile([P, M, KC], F32, tag="redsink")
                        nc.scalar.activation(
                            red[:], a[:], Act.Copy, bias=0.0, scale=SCALES[idx], accum_out=dst
                        )

                chain(p2, p4, 0)  # lambda=+1: zeta 2,4,8 -> l = 0,1,2
                chain(m2, m4, 3)  # lambda=-1: zeta 2,4,8 -> l = 3,4,5

            # ---------- finish: sum chunks, store ----------
            res = pool.tile([P, L], F32, tag="res")
            nc.vector.tensor_reduce(
                res[:], acc[:].rearrange("p (l c) -> p l c", c=NCH), axis=mybir.AxisListType.X, op=Alu.add
            )
            nc.sync.dma_start(out.rearrange("b i l -> (b i) l"), res[:])

    nc.compile()
    return nc


def _get_nc():
    global _NC
    if _NC is None:
        _NC = _build()
    return _NC


def kernel(d_cutoff, d, atom_coordinates):
    d_cutoff = np.ascontiguousarray(d_cutoff, dtype=np.float32)
    d = np.ascontiguousarray(d, dtype=np.float32)
    atom_coordinates = np.ascontiguousarray(atom_coordinates, dtype=np.float32)
    nc = _get_nc()
    in_maps = []
    for core in range(NCORES):
        sl = slice(core * BPC, (core + 1) * BPC)
        in_maps.append(
            {
                "d_cutoff": d_cutoff[sl],
                "d": d[sl],
                "atom_coordinates": atom_coordinates[sl],
            }
        )
    res = run_bass_kernel_spmd(nc, in_maps, core_ids=list(range(NCORES)))
    return np.concatenate([res.results[i]["out"] for i in range(NCORES)], axis=0)


if __name__ == "__main__":
    rng = np.random.default_rng(0)
    inputs = {
        "d_cutoff": rng.uniform(0, 1, (B, M, M)).astype(np.float32),
        "d": rng.uniform(0, 1, (B, M, M)).astype(np.float32),
        "atom_coordinates": rng.standard_normal((B, M, 3)).astype(np.float32),
    }
    out = kernel(**inputs)
    print("kernel out shape:", out.shape, "sample:", out[0, 0])
